# revision 36
# baseline (speedup 1.0000x reference)
"""GCN layer kernel for Trainium2, SPMD over 8 NeuronCores.

Reference computation (all fp32):
    adj_hat = rownorm(adj + I)                      # [N, N]
    out     = adj_hat @ (X @ W) + bias              # X: [N, T, A]

Sharding: T (time) axis split across 8 cores; adj/W/bias replicated.

Per-core kernel (T_SH = 256 time steps):
  setup (once): load adj [m,n], PE-transpose to adjT [n,m], add I on the
    diagonal blocks, column-normalize via a ones-vector matmul + reciprocal
    + partition_broadcast; load W; build a partition-broadcast bias tile.
  per t: Y_t^T[a, m] = sum_nck matmul(lhsT=X_t[n,a], rhs=adjT_hat[n,m])
         (X's natural [n, (t a)] SBUF layout is exactly the stationary
         operand - no transposes anywhere in the hot loop)
         out_t[m, o] = matmul(lhsT=Y_t^T[a, m_half], rhs=W[a, o])  x2
         out_sb = out_psum + bias_bcast  (fused with the PSUM->SBUF copy)
"""

import os
import sys

import numpy as np

for _p in ("/opt/trn_rl_repo", "/root/.axon_site/_ro/trn_rl_repo"):
    if os.path.isdir(_p) and _p not in sys.path:
        sys.path.insert(0, _p)

import concourse.bass as bass
import concourse.mybir as mybir
import concourse.tile as tile
from concourse import bacc
from concourse.bass_utils import run_bass_kernel_spmd
from concourse.masks import make_identity

N_NODES = 256
N_TIMES = 2048
N_FEAT = 128
N_CORES = 8
T_SH = N_TIMES // N_CORES  # 256 time steps per core
P = 128  # partitions
NCH = N_NODES // P  # 2 node chunks

F32 = mybir.dt.float32


def _gcn_body(tc, out, x, adj, w, b, t_sh, tb, g1_f32r=True, g2_f32r=True):
    nc = tc.nc
    nblk = t_sh // tb
    F32R = mybir.dt.float32r
    # fp32r (fp32 truncated to 11 mantissa bits) streams at 1 cycle/col for
    # N>=256 vs fp32's 4 passes. GEMM1 (N=256) uses it; GEMM2 (N=128) stays
    # full fp32 - it is hidden under the DMA roofline anyway.
    g1_dt = F32R if g1_f32r else F32
    g2_dt = F32R if g2_f32r else F32

    from contextlib import ExitStack

    with ExitStack() as ctx:
        const = ctx.enter_context(tc.tile_pool(name="const", bufs=1))

        ident = const.tile([P, P], F32)
        make_identity(nc, ident)

        # W duplicated side by side: fp32r matmuls stream at 1 cycle/col
        # only for moving dims >= 256, so GEMM2 multiplies against [W | W]
        # (N=256) and the epilogue reads just the first 128 PSUM columns.
        w_sb = const.tile([P, 2, P], g2_dt)
        w_dup_ap = bass.AP(
            tensor=w.tensor, offset=w.offset, ap=[w.ap[0], [0, 2], w.ap[1]]
        )
        nc.sync.dma_start(out=w_sb, in_=w_dup_ap)

        # bias replicated across all 128 partitions (free dim = output feature)
        bias_bc = const.tile([P, N_FEAT], F32)
        bias_bcast_ap = bass.AP(
            tensor=b.tensor, offset=b.offset, ap=[[0, P], b.ap[0]]
        )
        nc.sync.dma_start(out=bias_bc, in_=bias_bcast_ap)

        # adjT_hat[n, m] = (adj[m, n] + I) / deg[m], n on partitions
        adjT = [
            const.tile([P, N_NODES], g1_dt, name=f"adjT{c}", tag=f"adjT{c}")
            for c in range(NCH)
        ]

        # Main-loop SBUF pools are created BEFORE the setup scratch pool so
        # their addresses don't alias it - otherwise the first X-tile DMAs
        # inherit a WAR dependency on the whole adjacency-setup chain and the
        # DMA queue sits idle for ~20us at kernel start.
        xp = ctx.enter_context(tc.tile_pool(name="xp", bufs=8))
        op = ctx.enter_context(tc.tile_pool(name="op", bufs=3))
        ysb = ctx.enter_context(tc.tile_pool(name="ysb", bufs=tb + 2))

        # [n, t, a] viewed as [n%128, n//128, t, a] so one 2MB DMA moves both
        # node chunks of a time block (bigger transfers amortize DMA fixed
        # costs; per-partition runs stay 8KB contiguous).
        x4 = x.rearrange("(c n) t a -> n c t a", n=P)
        out4 = out.rearrange("(c m) t a -> m c t a", m=P)

        def load_x(blk):
            t0 = blk * tb
            xtc = xp.tile(
                [P, NCH, tb, N_FEAT], g1_dt, name=f"x_{blk}", tag="x"
            )
            nc.sync.dma_start(out=xtc, in_=x4[:, :, t0 : t0 + tb, :])
            return xtc

        # adjT holds the UNnormalized (adj + I)^T; the 1/deg row scaling is
        # applied at the very end as a per-partition scalar, so GEMM1 only
        # waits on the 4 PE transposes (short setup critical path).
        r_m = [
            const.tile([P, 1], F32, name=f"r{mc}", tag=f"r{mc}")
            for mc in range(NCH)
        ]
        setup = ctx.enter_context(tc.tile_pool(name="setup", bufs=1))
        # the tiny adjacency loads are issued BEFORE the bulk X prefetch so
        # the setup chain isn't queued behind megabytes on the DMA ring
        a_sb = []
        for mc in range(NCH):
            a_t = setup.tile([P, N_NODES], F32, name=f"a{mc}", tag=f"a{mc}")
            nc.sync.dma_start(out=a_t, in_=adj[mc * P : (mc + 1) * P, :])
            a_sb.append(a_t)

        PF = 8  # prefetch depth (= xp bufs)
        prefetched = [load_x(blk) for blk in range(min(PF, nblk))]

        with tc.tile_pool(name="setup_ps", bufs=1, space="PSUM") as setup_ps:
            for nck in range(NCH):
                for mc in range(NCH):
                    tp = setup_ps.tile([P, P], F32, name="tp", tag="tp")
                    nc.tensor.transpose(
                        tp, a_sb[mc][:, nck * P : (nck + 1) * P], ident
                    )
                    dst = adjT[nck][:, mc * P : (mc + 1) * P]
                    if mc == nck:
                        nc.vector.tensor_add(dst, tp, ident)
                    else:
                        nc.vector.tensor_copy(dst, tp)
            # r[m] = 1 / (1 + sum_n adj[m, n]) straight off the natural
            # [m, n] layout - no transpose or broadcast needed.
            for mc in range(NCH):
                dg = setup.tile([P, 1], F32, name=f"dg{mc}", tag=f"dg{mc}")
                nc.vector.reduce_sum(dg, a_sb[mc], axis=mybir.AxisListType.X)
                nc.vector.tensor_scalar_add(dg, dg, 1.0)
                nc.vector.reciprocal(r_m[mc], dg)

        yps = ctx.enter_context(tc.tile_pool(name="yps", bufs=3, space="PSUM"))
        ops = ctx.enter_context(tc.tile_pool(name="ops", bufs=2, space="PSUM"))

        for blk in range(nblk):
            t0 = blk * tb
            # sliding-window prefetch: issue the load PF blocks ahead NOW,
            # before this block's store enters the in-order sync queue -
            # otherwise store(k) head-of-line blocks load(k+PF)
            if blk + PF < nblk:
                prefetched.append(load_x(blk + PF))
            xt = prefetched[blk]
            ot = op.tile(
                [P, NCH, tb, N_FEAT], F32, name=f"o_{blk}", tag="o"
            )
            # Phase 1: all aggregation matmuls of the block + PSUM->SBUF
            # copies (ACT). Keeping PE on back-to-back GEMM1s gives the
            # copies time to land before phase 2 consumes them, so the
            # in-order PE queue never stalls on the DVE/ACT engines.
            ys_list = []
            for ti in range(tb):
                ypt = yps.tile([P, N_NODES], F32, name="ypt", tag="y")
                for ck in range(NCH):
                    nc.tensor.matmul(
                        ypt,
                        xt[:, ck, ti, :],
                        adjT[ck],
                        start=(ck == 0),
                        stop=(ck == NCH - 1),
                    )
                ys = ysb.tile([P, N_NODES], g2_dt, name=f"ys{ti}", tag="ys")
                nc.scalar.copy(ys, ypt)
                ys_list.append(ys)
            # Phase 2: feature-transform matmuls + scale/bias epilogue (DVE)
            for ti in range(tb):
                for mc in range(NCH):
                    opt = ops.tile([P, 2 * N_FEAT], F32, name="opt", tag=f"op{mc}")
                    nc.tensor.matmul(
                        opt,
                        ys_list[ti][:, mc * P : (mc + 1) * P],
                        w_sb.rearrange("p c o -> p (c o)"),
                        start=True,
                        stop=True,
                    )
                    nc.vector.scalar_tensor_tensor(
                        out=ot[:, mc, ti, :],
                        in0=opt[:, 0:N_FEAT],
                        scalar=r_m[mc],
                        in1=bias_bc,
                        op0=mybir.AluOpType.mult,
                        op1=mybir.AluOpType.add,
                    )
            nc.scalar.dma_start(out=out4[:, :, t0 : t0 + tb, :], in_=ot)


def build(t_sh=T_SH, tb=8, g1_f32r=True, g2_f32r=True):
    """Build + compile the per-core Bass module."""
    nc = bacc.Bacc(
        "TRN2", target_bir_lowering=False, debug=False, num_devices=N_CORES
    )
    x_dt = mybir.dt.float32r if g1_f32r else F32
    x = nc.dram_tensor("node_feats", [N_NODES, t_sh, N_FEAT], x_dt, kind="ExternalInput").ap()
    adj = nc.dram_tensor("adj_matrix", [N_NODES, N_NODES], F32, kind="ExternalInput").ap()
    w_dt = mybir.dt.float32r if g2_f32r else F32
    w = nc.dram_tensor("weight", [N_FEAT, N_FEAT], w_dt, kind="ExternalInput").ap()
    b = nc.dram_tensor("bias", [N_FEAT], F32, kind="ExternalInput").ap()
    out = nc.dram_tensor("out", [N_NODES, t_sh, N_FEAT], F32, kind="ExternalOutput").ap()
    with tile.TileContext(nc) as tc:
        _gcn_body(tc, out, x, adj, w, b, t_sh, tb, g1_f32r=g1_f32r, g2_f32r=g2_f32r)
    nc.compile()
    return nc


_built_nc = None


def _get_nc():
    global _built_nc
    if _built_nc is None:
        _built_nc = build()
    return _built_nc


def _run(node_feats, adj_matrix, weight, bias, trace=False, tmpdir=None):
    nc = _get_nc()
    node_feats = np.ascontiguousarray(node_feats, dtype=np.float32)
    adj_matrix = np.ascontiguousarray(adj_matrix, dtype=np.float32)
    weight = np.ascontiguousarray(weight, dtype=np.float32)
    bias = np.ascontiguousarray(bias, dtype=np.float32)
    in_maps = [
        {
            "node_feats": np.ascontiguousarray(
                node_feats[:, c * T_SH : (c + 1) * T_SH, :]
            ),
            "adj_matrix": adj_matrix,
            "weight": weight,
            "bias": bias,
        }
        for c in range(N_CORES)
    ]
    res = run_bass_kernel_spmd(
        nc, in_maps, list(range(N_CORES)), trace=trace, tmpdir=tmpdir
    )
    out = np.concatenate(
        [res.results[c]["out"] for c in range(N_CORES)], axis=1
    )
    return out, res


def kernel(node_feats, adj_matrix, weight, bias):
    out, _ = _run(node_feats, adj_matrix, weight, bias)
    return out


# revision 37
# speedup vs baseline: 1.0103x; 1.0103x over previous
"""GCN layer kernel for Trainium2, SPMD over 8 NeuronCores.

Reference computation (all fp32):
    adj_hat = rownorm(adj + I)                      # [N, N]
    out     = adj_hat @ (X @ W) + bias              # X: [N, T, A]

Sharding: T (time) axis split across 8 cores; adj/W/bias replicated.

Per-core kernel (T_SH = 256 time steps):
  setup (once): load adj [m,n], PE-transpose to adjT [n,m], add I on the
    diagonal blocks, column-normalize via a ones-vector matmul + reciprocal
    + partition_broadcast; load W; build a partition-broadcast bias tile.
  per t: Y_t^T[a, m] = sum_nck matmul(lhsT=X_t[n,a], rhs=adjT_hat[n,m])
         (X's natural [n, (t a)] SBUF layout is exactly the stationary
         operand - no transposes anywhere in the hot loop)
         out_t[m, o] = matmul(lhsT=Y_t^T[a, m_half], rhs=W[a, o])  x2
         out_sb = out_psum + bias_bcast  (fused with the PSUM->SBUF copy)
"""

import os
import sys

import numpy as np

for _p in ("/opt/trn_rl_repo", "/root/.axon_site/_ro/trn_rl_repo"):
    if os.path.isdir(_p) and _p not in sys.path:
        sys.path.insert(0, _p)

import concourse.bass as bass
import concourse.mybir as mybir
import concourse.tile as tile
from concourse import bacc
from concourse.bass_utils import run_bass_kernel_spmd
from concourse.masks import make_identity

N_NODES = 256
N_TIMES = 2048
N_FEAT = 128
N_CORES = 8
T_SH = N_TIMES // N_CORES  # 256 time steps per core
P = 128  # partitions
NCH = N_NODES // P  # 2 node chunks

F32 = mybir.dt.float32


def _gcn_body(tc, out, x, adj, w, b, t_sh, tb, g1_f32r=True, g2_f32r=True):
    nc = tc.nc
    nblk = t_sh // tb
    F32R = mybir.dt.float32r
    # fp32r (fp32 truncated to 11 mantissa bits) streams at 1 cycle/col for
    # N>=256 vs fp32's 4 passes. GEMM1 (N=256) uses it; GEMM2 (N=128) stays
    # full fp32 - it is hidden under the DMA roofline anyway.
    g1_dt = F32R if g1_f32r else F32
    g2_dt = F32R if g2_f32r else F32

    from contextlib import ExitStack

    with ExitStack() as ctx:
        const = ctx.enter_context(tc.tile_pool(name="const", bufs=1))

        ident = const.tile([P, P], F32)
        make_identity(nc, ident)

        # W duplicated side by side: fp32r matmuls stream at 1 cycle/col
        # only for moving dims >= 256, so GEMM2 multiplies against [W | W]
        # (N=256) and the epilogue reads just the first 128 PSUM columns.
        w_sb = const.tile([P, 2, P], g2_dt)
        w_dup_ap = bass.AP(
            tensor=w.tensor, offset=w.offset, ap=[w.ap[0], [0, 2], w.ap[1]]
        )
        nc.sync.dma_start(out=w_sb, in_=w_dup_ap)

        # bias replicated across all 128 partitions (free dim = output feature)
        bias_bc = const.tile([P, N_FEAT], F32)
        bias_bcast_ap = bass.AP(
            tensor=b.tensor, offset=b.offset, ap=[[0, P], b.ap[0]]
        )
        nc.sync.dma_start(out=bias_bc, in_=bias_bcast_ap)

        # adjT_hat[n, m] = (adj[m, n] + I) / deg[m], n on partitions
        adjT = [
            const.tile([P, N_NODES], g1_dt, name=f"adjT{c}", tag=f"adjT{c}")
            for c in range(NCH)
        ]

        # Main-loop SBUF pools are created BEFORE the setup scratch pool so
        # their addresses don't alias it - otherwise the first X-tile DMAs
        # inherit a WAR dependency on the whole adjacency-setup chain and the
        # DMA queue sits idle for ~20us at kernel start.
        xp = ctx.enter_context(tc.tile_pool(name="xp", bufs=4))
        op = ctx.enter_context(tc.tile_pool(name="op", bufs=3))
        ysb = ctx.enter_context(tc.tile_pool(name="ysb", bufs=tb + 2))

        # [n, t, a] viewed as [n%128, n//128, t, a] so one 2MB DMA moves both
        # node chunks of a time block (bigger transfers amortize DMA fixed
        # costs; per-partition runs stay 8KB contiguous).
        x4 = x.rearrange("(c n) t a -> n c t a", n=P)
        out4 = out.rearrange("(c m) t a -> m c t a", m=P)

        def load_x(blk):
            t0 = blk * tb
            xtc = xp.tile(
                [P, NCH, tb, N_FEAT], g1_dt, name=f"x_{blk}", tag="x"
            )
            nc.sync.dma_start(out=xtc, in_=x4[:, :, t0 : t0 + tb, :])
            return xtc

        # adjT holds the UNnormalized (adj + I)^T; the 1/deg row scaling is
        # applied at the very end as a per-partition scalar, so GEMM1 only
        # waits on the 4 PE transposes (short setup critical path).
        r_m = [
            const.tile([P, 1], F32, name=f"r{mc}", tag=f"r{mc}")
            for mc in range(NCH)
        ]
        setup = ctx.enter_context(tc.tile_pool(name="setup", bufs=1))
        # the tiny adjacency loads are issued BEFORE the bulk X prefetch so
        # the setup chain isn't queued behind megabytes on the DMA ring
        a_sb = []
        for mc in range(NCH):
            a_t = setup.tile([P, N_NODES], F32, name=f"a{mc}", tag=f"a{mc}")
            nc.sync.dma_start(out=a_t, in_=adj[mc * P : (mc + 1) * P, :])
            a_sb.append(a_t)

        PF = 4  # prefetch depth (= xp bufs)
        prefetched = [load_x(blk) for blk in range(min(PF, nblk))]

        with tc.tile_pool(name="setup_ps", bufs=1, space="PSUM") as setup_ps:
            for nck in range(NCH):
                for mc in range(NCH):
                    tp = setup_ps.tile([P, P], F32, name="tp", tag="tp")
                    nc.tensor.transpose(
                        tp, a_sb[mc][:, nck * P : (nck + 1) * P], ident
                    )
                    dst = adjT[nck][:, mc * P : (mc + 1) * P]
                    if mc == nck:
                        nc.vector.tensor_add(dst, tp, ident)
                    else:
                        nc.vector.tensor_copy(dst, tp)
            # r[m] = 1 / (1 + sum_n adj[m, n]) straight off the natural
            # [m, n] layout - no transpose or broadcast needed.
            for mc in range(NCH):
                dg = setup.tile([P, 1], F32, name=f"dg{mc}", tag=f"dg{mc}")
                nc.vector.reduce_sum(dg, a_sb[mc], axis=mybir.AxisListType.X)
                nc.vector.tensor_scalar_add(dg, dg, 1.0)
                nc.vector.reciprocal(r_m[mc], dg)

        yps = ctx.enter_context(tc.tile_pool(name="yps", bufs=3, space="PSUM"))
        ops = ctx.enter_context(tc.tile_pool(name="ops", bufs=2, space="PSUM"))

        for blk in range(nblk):
            t0 = blk * tb
            # sliding-window prefetch: issue the load PF blocks ahead NOW,
            # before this block's store enters the in-order sync queue -
            # otherwise store(k) head-of-line blocks load(k+PF)
            if blk + PF < nblk:
                prefetched.append(load_x(blk + PF))
            xt = prefetched[blk]
            ot = op.tile(
                [P, NCH, tb, N_FEAT], F32, name=f"o_{blk}", tag="o"
            )
            # Phase 1: all aggregation matmuls of the block + PSUM->SBUF
            # copies (ACT). Keeping PE on back-to-back GEMM1s gives the
            # copies time to land before phase 2 consumes them, so the
            # in-order PE queue never stalls on the DVE/ACT engines.
            ys_list = []
            for ti in range(tb):
                ypt = yps.tile([P, N_NODES], F32, name="ypt", tag="y")
                for ck in range(NCH):
                    nc.tensor.matmul(
                        ypt,
                        xt[:, ck, ti, :],
                        adjT[ck],
                        start=(ck == 0),
                        stop=(ck == NCH - 1),
                    )
                ys = ysb.tile([P, N_NODES], g2_dt, name=f"ys{ti}", tag="ys")
                nc.scalar.copy(ys, ypt)
                ys_list.append(ys)
            # Phase 2: feature-transform matmuls + scale/bias epilogue (DVE)
            for ti in range(tb):
                for mc in range(NCH):
                    opt = ops.tile([P, 2 * N_FEAT], F32, name="opt", tag=f"op{mc}")
                    nc.tensor.matmul(
                        opt,
                        ys_list[ti][:, mc * P : (mc + 1) * P],
                        w_sb.rearrange("p c o -> p (c o)"),
                        start=True,
                        stop=True,
                    )
                    nc.vector.scalar_tensor_tensor(
                        out=ot[:, mc, ti, :],
                        in0=opt[:, 0:N_FEAT],
                        scalar=r_m[mc],
                        in1=bias_bc,
                        op0=mybir.AluOpType.mult,
                        op1=mybir.AluOpType.add,
                    )
            if blk >= nblk - 2:
                # tail taper: store in quarter-block chunks as epilogues
                # complete, so the final store isn't gated on the whole
                # block finishing
                q = tb // 4
                for qi in range(4):
                    nc.scalar.dma_start(
                        out=out4[:, :, t0 + qi * q : t0 + (qi + 1) * q, :],
                        in_=ot[:, :, qi * q : (qi + 1) * q, :],
                    )
            else:
                nc.scalar.dma_start(out=out4[:, :, t0 : t0 + tb, :], in_=ot)


def build(t_sh=T_SH, tb=16, g1_f32r=True, g2_f32r=True):
    """Build + compile the per-core Bass module."""
    nc = bacc.Bacc(
        "TRN2", target_bir_lowering=False, debug=False, num_devices=N_CORES
    )
    x_dt = mybir.dt.float32r if g1_f32r else F32
    x = nc.dram_tensor("node_feats", [N_NODES, t_sh, N_FEAT], x_dt, kind="ExternalInput").ap()
    adj = nc.dram_tensor("adj_matrix", [N_NODES, N_NODES], F32, kind="ExternalInput").ap()
    w_dt = mybir.dt.float32r if g2_f32r else F32
    w = nc.dram_tensor("weight", [N_FEAT, N_FEAT], w_dt, kind="ExternalInput").ap()
    b = nc.dram_tensor("bias", [N_FEAT], F32, kind="ExternalInput").ap()
    out = nc.dram_tensor("out", [N_NODES, t_sh, N_FEAT], F32, kind="ExternalOutput").ap()
    with tile.TileContext(nc) as tc:
        _gcn_body(tc, out, x, adj, w, b, t_sh, tb, g1_f32r=g1_f32r, g2_f32r=g2_f32r)
    nc.compile()
    return nc


_built_nc = None


def _get_nc():
    global _built_nc
    if _built_nc is None:
        _built_nc = build()
    return _built_nc


def _run(node_feats, adj_matrix, weight, bias, trace=False, tmpdir=None):
    nc = _get_nc()
    node_feats = np.ascontiguousarray(node_feats, dtype=np.float32)
    adj_matrix = np.ascontiguousarray(adj_matrix, dtype=np.float32)
    weight = np.ascontiguousarray(weight, dtype=np.float32)
    bias = np.ascontiguousarray(bias, dtype=np.float32)
    in_maps = [
        {
            "node_feats": np.ascontiguousarray(
                node_feats[:, c * T_SH : (c + 1) * T_SH, :]
            ),
            "adj_matrix": adj_matrix,
            "weight": weight,
            "bias": bias,
        }
        for c in range(N_CORES)
    ]
    res = run_bass_kernel_spmd(
        nc, in_maps, list(range(N_CORES)), trace=trace, tmpdir=tmpdir
    )
    out = np.concatenate(
        [res.results[c]["out"] for c in range(N_CORES)], axis=1
    )
    return out, res


def kernel(node_feats, adj_matrix, weight, bias):
    out, _ = _run(node_feats, adj_matrix, weight, bias)
    return out


# revision 38
# speedup vs baseline: 1.0477x; 1.0370x over previous
"""GCN layer kernel for Trainium2, SPMD over 8 NeuronCores.

Reference computation (all fp32):
    adj_hat = rownorm(adj + I)                      # [N, N]
    out     = adj_hat @ (X @ W) + bias              # X: [N, T, A]

Sharding: T (time) axis split across 8 cores; adj/W/bias replicated.

Per-core kernel (T_SH = 256 time steps, time blocks of tb=16):
  setup (once): load adj [m,n], PE-transpose to adjT_raw [n,m] (+I on the
    diagonal blocks); r[m] = 1/(1+rowsum adj) as a per-partition scalar;
    load [W|W] (duplicated) and a partition-broadcast bias tile.
  per t: Y_t^T[a, m] = sum_nck matmul(lhsT=X_t[n,a], rhs=adjT_raw[n,m])
         (X's natural [n, (t a)] SBUF layout is exactly the stationary
         operand - no transposes anywhere in the hot loop; fp32r, N=256)
         psum[m, 256] = matmul(lhsT=Y_t^T[a, m_half], rhs=[W|W])  x2
         (fp32r needs moving dim >= 256 for 1 cyc/col - W is duplicated
         and only the first 128 PSUM columns are consumed)
         out_sb = r[m] * psum[:, :128] + bias_bcast  (one DVE op)
  Loads on the sync HWDGE ring, stores on the scalar ring; X prefetched
  4 blocks deep with loads emitted before stores (in-order queues).
"""

import os
import sys

import numpy as np

for _p in ("/opt/trn_rl_repo", "/root/.axon_site/_ro/trn_rl_repo"):
    if os.path.isdir(_p) and _p not in sys.path:
        sys.path.insert(0, _p)

import concourse.bass as bass
import concourse.mybir as mybir
import concourse.tile as tile
from concourse import bacc
from concourse.bass_utils import run_bass_kernel_spmd
from concourse.masks import make_identity

N_NODES = 256
N_TIMES = 2048
N_FEAT = 128
N_CORES = 8
T_SH = N_TIMES // N_CORES  # 256 time steps per core
P = 128  # partitions
NCH = N_NODES // P  # 2 node chunks

F32 = mybir.dt.float32


def _gcn_body(tc, out, x, adj, w, b, t_sh, tb, g1_f32r=True, g2_f32r=True):
    nc = tc.nc
    nblk = t_sh // tb
    F32R = mybir.dt.float32r
    # fp32r (fp32 truncated to 11 mantissa bits) streams at 1 cycle/col for
    # N>=256 vs fp32's 4 passes. GEMM1 (N=256) uses it; GEMM2 (N=128) stays
    # full fp32 - it is hidden under the DMA roofline anyway.
    g1_dt = F32R if g1_f32r else F32
    g2_dt = F32R if g2_f32r else F32

    from contextlib import ExitStack

    with ExitStack() as ctx:
        const = ctx.enter_context(tc.tile_pool(name="const", bufs=1))

        ident = const.tile([P, P], F32)
        make_identity(nc, ident)

        # W duplicated side by side: fp32r matmuls stream at 1 cycle/col
        # only for moving dims >= 256, so GEMM2 multiplies against [W | W]
        # (N=256) and the epilogue reads just the first 128 PSUM columns.
        w_sb = const.tile([P, 2, P], g2_dt)
        w_dup_ap = bass.AP(
            tensor=w.tensor, offset=w.offset, ap=[w.ap[0], [0, 2], w.ap[1]]
        )
        nc.sync.dma_start(out=w_sb, in_=w_dup_ap)

        # bias replicated across all 128 partitions (free dim = output feature)
        bias_bc = const.tile([P, N_FEAT], F32)
        bias_bcast_ap = bass.AP(
            tensor=b.tensor, offset=b.offset, ap=[[0, P], b.ap[0]]
        )
        nc.sync.dma_start(out=bias_bc, in_=bias_bcast_ap)

        # adjT_hat[n, m] = (adj[m, n] + I) / deg[m], n on partitions
        adjT = [
            const.tile([P, N_NODES], g1_dt, name=f"adjT{c}", tag=f"adjT{c}")
            for c in range(NCH)
        ]

        # Main-loop SBUF pools are created BEFORE the setup scratch pool so
        # their addresses don't alias it - otherwise the first X-tile DMAs
        # inherit a WAR dependency on the whole adjacency-setup chain and the
        # DMA queue sits idle for ~20us at kernel start.
        xp = ctx.enter_context(tc.tile_pool(name="xp", bufs=4))
        op = ctx.enter_context(tc.tile_pool(name="op", bufs=3))
        ysb = ctx.enter_context(tc.tile_pool(name="ysb", bufs=tb + 2))

        # [n, t, a] viewed as [n%128, n//128, t, a] so one 2MB DMA moves both
        # node chunks of a time block (bigger transfers amortize DMA fixed
        # costs; per-partition runs stay 8KB contiguous).
        x4 = x.rearrange("(c n) t a -> n c t a", n=P)
        out4 = out.rearrange("(c m) t a -> m c t a", m=P)

        def load_x(blk):
            t0 = blk * tb
            xtc = xp.tile(
                [P, NCH, tb, N_FEAT], g1_dt, name=f"x_{blk}", tag="x"
            )
            nc.sync.dma_start(out=xtc, in_=x4[:, :, t0 : t0 + tb, :])
            return xtc

        # adjT holds the UNnormalized (adj + I)^T; the 1/deg row scaling is
        # applied at the very end as a per-partition scalar, so GEMM1 only
        # waits on the 4 PE transposes (short setup critical path).
        r_m = [
            const.tile([P, 1], F32, name=f"r{mc}", tag=f"r{mc}")
            for mc in range(NCH)
        ]
        setup = ctx.enter_context(tc.tile_pool(name="setup", bufs=1))
        # the tiny adjacency loads are issued BEFORE the bulk X prefetch so
        # the setup chain isn't queued behind megabytes on the DMA ring
        a_sb = []
        for mc in range(NCH):
            a_t = setup.tile([P, N_NODES], F32, name=f"a{mc}", tag=f"a{mc}")
            nc.sync.dma_start(out=a_t, in_=adj[mc * P : (mc + 1) * P, :])
            a_sb.append(a_t)

        PF = 4  # prefetch depth (= xp bufs)
        prefetched = [load_x(blk) for blk in range(min(PF, nblk))]

        with tc.tile_pool(name="setup_ps", bufs=1, space="PSUM") as setup_ps:
            for nck in range(NCH):
                for mc in range(NCH):
                    tp = setup_ps.tile([P, P], F32, name="tp", tag="tp")
                    nc.tensor.transpose(
                        tp, a_sb[mc][:, nck * P : (nck + 1) * P], ident
                    )
                    dst = adjT[nck][:, mc * P : (mc + 1) * P]
                    if mc == nck:
                        nc.vector.tensor_add(dst, tp, ident)
                    else:
                        nc.vector.tensor_copy(dst, tp)
            # r[m] = 1 / (1 + sum_n adj[m, n]) straight off the natural
            # [m, n] layout - no transpose or broadcast needed.
            for mc in range(NCH):
                dg = setup.tile([P, 1], F32, name=f"dg{mc}", tag=f"dg{mc}")
                nc.vector.reduce_sum(dg, a_sb[mc], axis=mybir.AxisListType.X)
                nc.vector.tensor_scalar_add(dg, dg, 1.0)
                nc.vector.reciprocal(r_m[mc], dg)

        yps = ctx.enter_context(tc.tile_pool(name="yps", bufs=3, space="PSUM"))
        ops = ctx.enter_context(tc.tile_pool(name="ops", bufs=2, space="PSUM"))

        for blk in range(nblk):
            t0 = blk * tb
            # sliding-window prefetch: issue the load PF blocks ahead NOW,
            # before this block's store enters the in-order sync queue -
            # otherwise store(k) head-of-line blocks load(k+PF)
            if blk + PF < nblk:
                prefetched.append(load_x(blk + PF))
            xt = prefetched[blk]
            ot = op.tile(
                [P, NCH, tb, N_FEAT], F32, name=f"o_{blk}", tag="o"
            )
            # Phase 1: all aggregation matmuls of the block + PSUM->SBUF
            # copies (ACT). Keeping PE on back-to-back GEMM1s gives the
            # copies time to land before phase 2 consumes them, so the
            # in-order PE queue never stalls on the DVE/ACT engines.
            ys_list = []
            for ti in range(tb):
                ypt = yps.tile([P, N_NODES], F32, name="ypt", tag="y")
                for ck in range(NCH):
                    nc.tensor.matmul(
                        ypt,
                        xt[:, ck, ti, :],
                        adjT[ck],
                        start=(ck == 0),
                        stop=(ck == NCH - 1),
                    )
                ys = ysb.tile([P, N_NODES], g2_dt, name=f"ys{ti}", tag="ys")
                nc.scalar.copy(ys, ypt)
                ys_list.append(ys)
            # Phase 2: feature-transform matmuls + scale/bias epilogue (DVE)
            for ti in range(tb):
                for mc in range(NCH):
                    opt = ops.tile([P, 2 * N_FEAT], F32, name="opt", tag=f"op{mc}")
                    nc.tensor.matmul(
                        opt,
                        ys_list[ti][:, mc * P : (mc + 1) * P],
                        w_sb.rearrange("p c o -> p (c o)"),
                        start=True,
                        stop=True,
                    )
                    nc.vector.scalar_tensor_tensor(
                        out=ot[:, mc, ti, :],
                        in0=opt[:, 0:N_FEAT],
                        scalar=r_m[mc],
                        in1=bias_bc,
                        op0=mybir.AluOpType.mult,
                        op1=mybir.AluOpType.add,
                    )
            nc.scalar.dma_start(out=out4[:, :, t0 : t0 + tb, :], in_=ot)


def build(t_sh=T_SH, tb=16, g1_f32r=True, g2_f32r=True):
    """Build + compile the per-core Bass module."""
    nc = bacc.Bacc(
        "TRN2", target_bir_lowering=False, debug=False, num_devices=N_CORES
    )
    x_dt = mybir.dt.float32r if g1_f32r else F32
    x = nc.dram_tensor("node_feats", [N_NODES, t_sh, N_FEAT], x_dt, kind="ExternalInput").ap()
    adj = nc.dram_tensor("adj_matrix", [N_NODES, N_NODES], F32, kind="ExternalInput").ap()
    w_dt = mybir.dt.float32r if g2_f32r else F32
    w = nc.dram_tensor("weight", [N_FEAT, N_FEAT], w_dt, kind="ExternalInput").ap()
    b = nc.dram_tensor("bias", [N_FEAT], F32, kind="ExternalInput").ap()
    out = nc.dram_tensor("out", [N_NODES, t_sh, N_FEAT], F32, kind="ExternalOutput").ap()
    with tile.TileContext(nc) as tc:
        _gcn_body(tc, out, x, adj, w, b, t_sh, tb, g1_f32r=g1_f32r, g2_f32r=g2_f32r)
    nc.compile()
    return nc


_built_nc = None


def _get_nc():
    global _built_nc
    if _built_nc is None:
        _built_nc = build()
    return _built_nc


def _run(node_feats, adj_matrix, weight, bias, trace=False, tmpdir=None):
    nc = _get_nc()
    node_feats = np.ascontiguousarray(node_feats, dtype=np.float32)
    adj_matrix = np.ascontiguousarray(adj_matrix, dtype=np.float32)
    weight = np.ascontiguousarray(weight, dtype=np.float32)
    bias = np.ascontiguousarray(bias, dtype=np.float32)
    in_maps = [
        {
            "node_feats": np.ascontiguousarray(
                node_feats[:, c * T_SH : (c + 1) * T_SH, :]
            ),
            "adj_matrix": adj_matrix,
            "weight": weight,
            "bias": bias,
        }
        for c in range(N_CORES)
    ]
    res = run_bass_kernel_spmd(
        nc, in_maps, list(range(N_CORES)), trace=trace, tmpdir=tmpdir
    )
    out = np.concatenate(
        [res.results[c]["out"] for c in range(N_CORES)], axis=1
    )
    return out, res


def kernel(node_feats, adj_matrix, weight, bias):
    out, _ = _run(node_feats, adj_matrix, weight, bias)
    return out
